# revision 2
# baseline (speedup 1.0000x reference)
"""Trainium2 Bass kernel: matvec, bf16 cast-on-DMA, tapered tail (v6.2).

scores = encoder_out[16384, 4096] @ decoder_hidden[-1][4096] -> [16384]
Sharding: encoder_out row-wise across 8 cores (2048 rows each),
decoder_hidden replicated; no cross-core communication.

Structure (per core, 32 MB fp32 read -> 16 MB bf16 in SBUF):
  - SWDGE (gpsimd) dma_start stream with fp32 -> bf16 cast in flight.
    SWDGE also avoids the HWDGE engine-15 descriptor-rate penalty in
    most runs (engine 15 has an environmental slow mode either way).
  - t broadcast to all 128 partitions with cast, also SWDGE (1 MB).
  - Blocks b0..b13: [128, 4096] tiles (rows n*128+p), 8 buffer slots.
  - Tail taper: b14 as two [128, 2048] halves reduced by ACT;
    b15 as four [128, 1024] quarters multiplied and reduced by DVE
    (reduce_sum into [128,1] scratch, 3 adds combine) so ACT and DVE
    drain the tail in parallel.
  - DVE tensor_mul in place (bf16), ACT Copy+accum_out does the row
    sums (fp32 accumulator).
  - Stores: sc[:, 0:12] early (descriptor generation overlaps the
    stream), sc[:, 12:17] behind the last reduce only.
  - Slot sems are reused for the tail tiles (each sem's final transfer
    makes the cumulative wait exact): 13 semaphores total keeps the
    preamble sem-init short.

Output sc [128, 17]: cols 0..13 = b0..b13 scores; block14 score =
sc[:,14] + sc[:,15] (the two halves); block15 score = sc[:,16].

Accuracy: enc and t are rounded to bf16 (products bf16, fp32
accumulate) -> max rel err ~3.4e-3, well under the 2e-2 gate.
"""

import numpy as np

S, H, L = 16384, 4096, 2
N_CORES = 8
S_LOC = S // N_CORES        # 2048
P = 128
N_BLOCKS = S_LOC // P       # 16
NBUF = 8
HH = H // 2                 # 2048
QW = H // 4                 # 1024

_NC_CACHE = {}
LAST_RESULT = None


def _build_nc():
    import concourse.bass as bass
    from concourse import mybir

    f32 = mybir.dt.float32
    bf16 = mybir.dt.bfloat16

    nc = bass.Bass(trn_type="TRN2")
    enc = nc.dram_tensor("enc", [S_LOC, H], f32, kind="ExternalInput")
    dec = nc.dram_tensor("dec", [L, H], f32, kind="ExternalInput")
    out = nc.dram_tensor("out", [P, 17], f32, kind="ExternalOutput")

    enc_r = enc.rearrange("(n p) h -> n p h", p=P)

    from contextlib import ExitStack

    with ExitStack() as ctx:
        tb = ctx.enter_context(nc.sbuf_tensor("tb", [P, H], bf16))
        ebufs = [
            ctx.enter_context(nc.sbuf_tensor(f"ebuf{i}", [P, H], bf16))
            for i in range(NBUF)
        ]
        junk = ctx.enter_context(nc.sbuf_tensor("junk", [P, H], bf16))
        sc = ctx.enter_context(nc.sbuf_tensor("sc", [P, 17], f32))
        scb = [
            ctx.enter_context(nc.sbuf_tensor(f"scb{k}", [P, 1], f32))
            for k in range(4)
        ]
        tb_sem = ctx.enter_context(nc.semaphore("tb_sem"))
        esems = [ctx.enter_context(nc.semaphore(f"esem{i}")) for i in range(NBUF)]
        hsems = [ctx.enter_context(nc.semaphore(f"hsem{i}")) for i in range(2)]
        qsems = [ctx.enter_context(nc.semaphore(f"qsem{i}")) for i in range(4)]
        mul_sem = ctx.enter_context(nc.semaphore("mul_sem"))
        red_sem = ctx.enter_context(nc.semaphore("red_sem"))
        qred_sem = ctx.enter_context(nc.semaphore("qred_sem"))
        store_sem = ctx.enter_context(nc.semaphore("store_sem"))
        block = ctx.enter_context(nc.Block())

        @block.sync
        def _(sync):
            # bulk store early: HWDGE descriptor generation (~128 descs)
            # overlaps the stream; only cols 12:17 wait for the tail
            sync.wait_ge(red_sem, 12)
            sync.dma_start(out[:, 0:12], sc[:, 0:12]).then_inc(store_sem, 16)
            sync.wait_ge(red_sem, 16)
            sync.wait_ge(qred_sem, 1)
            sync.dma_start(out[:, 12:17], sc[:, 12:17]).then_inc(store_sem, 16)
            sync.wait_ge(store_sem, 32)

        @block.gpsimd
        def _(gpsimd):
            # t broadcast with fp32 -> bf16 cast (1 MB written)
            gpsimd.dma_start(
                tb[:], dec[L - 1 : L, :].to_broadcast((P, H))
            ).then_inc(tb_sem, 16)
            # b0..b13 full tiles
            for i in range(N_BLOCKS - 2):
                if i >= NBUF:
                    gpsimd.wait_ge(red_sem, i - NBUF + 1)
                gpsimd.dma_start(ebufs[i % NBUF][:], enc_r[i]).then_inc(
                    esems[i % NBUF], 16
                )
            # b14 halves into slot 6 (b6's ACT frees it)
            gpsimd.wait_ge(red_sem, 7)
            gpsimd.dma_start(
                ebufs[6][:, 0:HH], enc_r[14, :, 0:HH]
            ).then_inc(hsems[0], 16)
            gpsimd.dma_start(
                ebufs[6][:, HH:H], enc_r[14, :, HH:H]
            ).then_inc(hsems[1], 16)
            # b15 quarters into slot 7 (b7's ACT frees it)
            gpsimd.wait_ge(red_sem, 8)
            for k in range(4):
                gpsimd.dma_start(
                    ebufs[7][:, k * QW : (k + 1) * QW],
                    enc_r[15, :, k * QW : (k + 1) * QW],
                ).then_inc(qsems[k], 16)

        @block.vector
        def _(vector):
            vector.wait_ge(tb_sem, 16)
            for n in range(N_BLOCKS - 2):
                vector.wait_ge(esems[n % NBUF], 16 * (n // NBUF + 1))
                eb = ebufs[n % NBUF][:]
                nc.vector.tensor_mul(eb, eb, tb[:]).then_inc(mul_sem, 1)
            # b14 halves (ACT reduces them)
            vector.wait_ge(hsems[0], 16)
            nc.vector.tensor_mul(
                ebufs[6][:, 0:HH], ebufs[6][:, 0:HH], tb[:, 0:HH]
            ).then_inc(mul_sem, 1)
            vector.wait_ge(hsems[1], 16)
            nc.vector.tensor_mul(
                ebufs[6][:, HH:H], ebufs[6][:, HH:H], tb[:, HH:H]
            ).then_inc(mul_sem, 1)
            # b15 quarters: DVE multiplies AND reduces (ACT is busy with
            # the halves); 3 adds combine the 4 partials
            e7 = ebufs[7]
            for k in range(4):
                vector.wait_ge(qsems[k], 16)
                nc.vector.tensor_mul(
                    e7[:, k * QW : (k + 1) * QW],
                    e7[:, k * QW : (k + 1) * QW],
                    tb[:, k * QW : (k + 1) * QW],
                )
                nc.vector.reduce_sum(
                    out=scb[k][:],
                    in_=e7[:, k * QW : (k + 1) * QW],
                    axis=mybir.AxisListType.X,
                )
            nc.vector.tensor_add(scb[0][:], scb[0][:], scb[1][:])
            nc.vector.tensor_add(scb[2][:], scb[2][:], scb[3][:])
            # two junk spacer ops: the DVE pipeline has no RAW interlock at
            # 1-instruction distance (~70 ns); the final add must read
            # scb[2] at distance >= 3 (measured: distance-1 reads stale)
            nc.vector.tensor_add(scb[1][:], scb[1][:], scb[3][:])
            nc.vector.tensor_add(scb[3][:], scb[3][:], scb[1][:])
            # write sc[:, 16] only after ACT has finished its sc writes
            vector.wait_ge(red_sem, 16)
            nc.vector.tensor_add(
                sc[:, 16:17], scb[0][:], scb[2][:]
            ).then_inc(qred_sem, 1)

        @block.scalar
        def _(scalar):
            # warm the ACT function table while idle
            nc.scalar.activation(
                out=junk[0:1, 0:1],
                in_=junk[0:1, 0:1],
                func=mybir.ActivationFunctionType.Copy,
            )
            for n in range(N_BLOCKS - 2):
                scalar.wait_ge(mul_sem, n + 1)
                nc.scalar.activation(
                    out=junk[:],
                    in_=ebufs[n % NBUF][:],
                    func=mybir.ActivationFunctionType.Copy,
                    accum_out=sc[:, n : n + 1],
                ).then_inc(red_sem, 1)
            # b14 halves -> sc cols 14, 15
            scalar.wait_ge(mul_sem, 15)
            nc.scalar.activation(
                out=junk[:, 0:HH],
                in_=ebufs[6][:, 0:HH],
                func=mybir.ActivationFunctionType.Copy,
                accum_out=sc[:, 14:15],
            ).then_inc(red_sem, 1)
            scalar.wait_ge(mul_sem, 16)
            nc.scalar.activation(
                out=junk[:, HH:H],
                in_=ebufs[6][:, HH:H],
                func=mybir.ActivationFunctionType.Copy,
                accum_out=sc[:, 15:16],
            ).then_inc(red_sem, 1)

    return nc


def kernel(encoder_out: np.ndarray, decoder_hidden: np.ndarray) -> np.ndarray:
    global LAST_RESULT
    from concourse.bass_utils import run_bass_kernel_spmd

    encoder_out = np.ascontiguousarray(np.asarray(encoder_out, dtype=np.float32))
    decoder_hidden = np.ascontiguousarray(np.asarray(decoder_hidden, dtype=np.float32))

    if "nc" not in _NC_CACHE:
        _NC_CACHE["nc"] = _build_nc()
    nc = _NC_CACHE["nc"]

    in_maps = [
        {"enc": encoder_out[c * S_LOC : (c + 1) * S_LOC], "dec": decoder_hidden}
        for c in range(N_CORES)
    ]
    res = run_bass_kernel_spmd(nc, in_maps, core_ids=list(range(N_CORES)))
    LAST_RESULT = res

    parts = []
    for r in res.results:
        sc = np.asarray(r["out"])  # [128, 17]
        blk = np.concatenate(
            [
                sc[:, 0:14],
                (sc[:, 14] + sc[:, 15])[:, None],   # block 14
                sc[:, 16:17],                        # block 15
            ],
            axis=1,
        )  # [128, 16]
        parts.append(blk.T.reshape(-1))
    return np.concatenate(parts).astype(np.float32)


# revision 3
# speedup vs baseline: 1.2146x; 1.2146x over previous
"""Trainium2 Bass kernel: matvec, bf16 cast-on-DMA, PE t-broadcast, tapered tail (v8).

scores = encoder_out[16384, 4096] @ decoder_hidden[-1][4096] -> [16384]
Sharding: encoder_out row-wise across 8 cores (2048 rows each),
decoder_hidden replicated; no cross-core communication.

Structure (per core, 32 MB fp32 read -> 16 MB bf16 in SBUF):
  - SWDGE (gpsimd) dma_start stream with fp32 -> bf16 cast in flight.
    SWDGE also avoids the HWDGE engine-15 descriptor-rate penalty in
    most runs (engine 15 has an environmental slow mode either way).
  - t broadcast to all 128 partitions with cast, also SWDGE (1 MB).
  - Blocks b0..b13: [128, 4096] tiles (rows n*128+p), 8 buffer slots.
  - Tail taper: b14 as two [128, 2048] halves reduced by ACT;
    b15 as four [128, 1024] quarters multiplied and reduced by DVE
    (reduce_sum into [128,1] scratch, 3 adds combine) so ACT and DVE
    drain the tail in parallel.
  - DVE tensor_mul in place (bf16), ACT Copy+accum_out does the row
    sums (fp32 accumulator).
  - Stores: sc[:, 0:12] early (descriptor generation overlaps the
    stream), sc[:, 12:17] behind the last reduce only.
  - Slot sems are reused for the tail tiles (each sem's final transfer
    makes the cumulative wait exact): 13 semaphores total keeps the
    preamble sem-init short.

Output sc [128, 17]: cols 0..13 = b0..b13 scores; block14 score =
sc[:,14] + sc[:,15] (the two halves); block15 score = sc[:,16].

Accuracy: enc and t are rounded to bf16 (products bf16, fp32
accumulate) -> max rel err ~3.4e-3, well under the 2e-2 gate.
"""

import numpy as np

S, H, L = 16384, 4096, 2
N_CORES = 8
S_LOC = S // N_CORES        # 2048
P = 128
N_BLOCKS = S_LOC // P       # 16
NBUF = 8
HH = H // 2                 # 2048
QW = H // 4                 # 1024

_NC_CACHE = {}
LAST_RESULT = None


def _build_nc():
    import concourse.bass as bass
    from concourse import mybir

    f32 = mybir.dt.float32
    bf16 = mybir.dt.bfloat16

    nc = bass.Bass(trn_type="TRN2")
    enc = nc.dram_tensor("enc", [S_LOC, H], f32, kind="ExternalInput")
    dec = nc.dram_tensor("dec", [L, H], f32, kind="ExternalInput")
    out = nc.dram_tensor("out", [P, 17], f32, kind="ExternalOutput")

    enc_r = enc.rearrange("(n p) h -> n p h", p=P)

    from contextlib import ExitStack

    with ExitStack() as ctx:
        t16 = ctx.enter_context(nc.sbuf_tensor("t16", [1, H], bf16))
        ones = ctx.enter_context(nc.sbuf_tensor("ones", [1, P], bf16))
        tb = ctx.enter_context(nc.psum_tensor("tbps", [P, H], f32))
        ebufs = [
            ctx.enter_context(nc.sbuf_tensor(f"ebuf{i}", [P, H], bf16))
            for i in range(NBUF)
        ]
        junk = ctx.enter_context(nc.sbuf_tensor("junk", [P, H], bf16))
        sc = ctx.enter_context(nc.sbuf_tensor("sc", [P, 17], f32))
        scb = [
            ctx.enter_context(nc.sbuf_tensor(f"scb{k}", [P, 1], f32))
            for k in range(4)
        ]
        t_sem = ctx.enter_context(nc.semaphore("t_sem"))
        ones_sem = ctx.enter_context(nc.semaphore("ones_sem"))
        pe_sem = ctx.enter_context(nc.semaphore("pe_sem"))
        esems = [ctx.enter_context(nc.semaphore(f"esem{i}")) for i in range(NBUF)]
        hsems = [ctx.enter_context(nc.semaphore(f"hsem{i}")) for i in range(2)]
        qsems = [ctx.enter_context(nc.semaphore(f"qsem{i}")) for i in range(4)]
        mul_sem = ctx.enter_context(nc.semaphore("mul_sem"))
        red_sem = ctx.enter_context(nc.semaphore("red_sem"))
        qred_sem = ctx.enter_context(nc.semaphore("qred_sem"))
        store_sem = ctx.enter_context(nc.semaphore("store_sem"))
        block = ctx.enter_context(nc.Block())

        @block.sync
        def _(sync):
            # bulk store early: HWDGE descriptor generation (~128 descs)
            # overlaps the stream; only cols 12:17 wait for the tail
            sync.wait_ge(red_sem, 12)
            sync.dma_start(out[:, 0:12], sc[:, 0:12]).then_inc(store_sem, 16)
            sync.wait_ge(red_sem, 16)
            sync.wait_ge(qred_sem, 1)
            sync.dma_start(out[:, 12:17], sc[:, 12:17]).then_inc(store_sem, 16)
            sync.wait_ge(store_sem, 32)

        @block.gpsimd
        def _(gpsimd):
            # t into one partition with fp32 -> bf16 cast (8 KB written);
            # the idle PE broadcasts it to all 128 partitions via a
            # ones-vector matmul into PSUM -- saves ~5-6 us of broadcast
            # descriptors on every DMA engine
            gpsimd.memset(ones[:], 1.0).then_inc(ones_sem, 1)
            gpsimd.dma_start(t16[:], dec[L - 1 : L, :]).then_inc(t_sem, 16)
            # b0..b13 full tiles
            for i in range(N_BLOCKS - 2):
                if i >= NBUF:
                    gpsimd.wait_ge(red_sem, i - NBUF + 1)
                gpsimd.dma_start(ebufs[i % NBUF][:], enc_r[i]).then_inc(
                    esems[i % NBUF], 16
                )
            # b14 halves into slot 6 (b6's ACT frees it)
            gpsimd.wait_ge(red_sem, 7)
            gpsimd.dma_start(
                ebufs[6][:, 0:HH], enc_r[14, :, 0:HH]
            ).then_inc(hsems[0], 16)
            gpsimd.dma_start(
                ebufs[6][:, HH:H], enc_r[14, :, HH:H]
            ).then_inc(hsems[1], 16)
            # b15 quarters into slot 7 (b7's ACT frees it)
            gpsimd.wait_ge(red_sem, 8)
            for k in range(4):
                gpsimd.dma_start(
                    ebufs[7][:, k * QW : (k + 1) * QW],
                    enc_r[15, :, k * QW : (k + 1) * QW],
                ).then_inc(qsems[k], 16)

        @block.tensor
        def _(tensor):
            tensor.wait_ge(ones_sem, 1)
            tensor.wait_ge(t_sem, 16)
            for j in range(8):
                mm = nc.tensor.matmul(
                    tb[:, j * 512 : (j + 1) * 512],
                    ones[:],
                    t16[:, j * 512 : (j + 1) * 512],
                    start=True,
                    stop=True,
                )
            mm.then_inc(pe_sem, 1)

        @block.vector
        def _(vector):
            vector.wait_ge(pe_sem, 1)
            for n in range(N_BLOCKS - 2):
                vector.wait_ge(esems[n % NBUF], 16 * (n // NBUF + 1))
                eb = ebufs[n % NBUF][:]
                nc.vector.tensor_mul(eb, eb, tb[:]).then_inc(mul_sem, 1)
            # b14 halves (ACT reduces them)
            vector.wait_ge(hsems[0], 16)
            nc.vector.tensor_mul(
                ebufs[6][:, 0:HH], ebufs[6][:, 0:HH], tb[:, 0:HH]
            ).then_inc(mul_sem, 1)
            vector.wait_ge(hsems[1], 16)
            nc.vector.tensor_mul(
                ebufs[6][:, HH:H], ebufs[6][:, HH:H], tb[:, HH:H]
            ).then_inc(mul_sem, 1)
            # b15 quarters: DVE multiplies AND reduces (ACT is busy with
            # the halves); 3 adds combine the 4 partials
            e7 = ebufs[7]
            for k in range(4):
                vector.wait_ge(qsems[k], 16)
                nc.vector.tensor_mul(
                    e7[:, k * QW : (k + 1) * QW],
                    e7[:, k * QW : (k + 1) * QW],
                    tb[:, k * QW : (k + 1) * QW],
                )
                nc.vector.reduce_sum(
                    out=scb[k][:],
                    in_=e7[:, k * QW : (k + 1) * QW],
                    axis=mybir.AxisListType.X,
                )
            nc.vector.tensor_add(scb[0][:], scb[0][:], scb[1][:])
            nc.vector.tensor_add(scb[2][:], scb[2][:], scb[3][:])
            # two junk spacer ops: the DVE pipeline has no RAW interlock at
            # 1-instruction distance (~70 ns); the final add must read
            # scb[2] at distance >= 3 (measured: distance-1 reads stale)
            nc.vector.tensor_add(scb[1][:], scb[1][:], scb[3][:])
            nc.vector.tensor_add(scb[3][:], scb[3][:], scb[1][:])
            # write sc[:, 16] only after ACT has finished its sc writes
            vector.wait_ge(red_sem, 16)
            nc.vector.tensor_add(
                sc[:, 16:17], scb[0][:], scb[2][:]
            ).then_inc(qred_sem, 1)

        @block.scalar
        def _(scalar):
            # warm the ACT function table while idle
            nc.scalar.activation(
                out=junk[0:1, 0:1],
                in_=junk[0:1, 0:1],
                func=mybir.ActivationFunctionType.Copy,
            )
            for n in range(N_BLOCKS - 2):
                scalar.wait_ge(mul_sem, n + 1)
                nc.scalar.activation(
                    out=junk[:],
                    in_=ebufs[n % NBUF][:],
                    func=mybir.ActivationFunctionType.Copy,
                    accum_out=sc[:, n : n + 1],
                ).then_inc(red_sem, 1)
            # b14 halves -> sc cols 14, 15
            scalar.wait_ge(mul_sem, 15)
            nc.scalar.activation(
                out=junk[:, 0:HH],
                in_=ebufs[6][:, 0:HH],
                func=mybir.ActivationFunctionType.Copy,
                accum_out=sc[:, 14:15],
            ).then_inc(red_sem, 1)
            scalar.wait_ge(mul_sem, 16)
            nc.scalar.activation(
                out=junk[:, HH:H],
                in_=ebufs[6][:, HH:H],
                func=mybir.ActivationFunctionType.Copy,
                accum_out=sc[:, 15:16],
            ).then_inc(red_sem, 1)

    return nc


def kernel(encoder_out: np.ndarray, decoder_hidden: np.ndarray) -> np.ndarray:
    global LAST_RESULT
    from concourse.bass_utils import run_bass_kernel_spmd

    encoder_out = np.ascontiguousarray(np.asarray(encoder_out, dtype=np.float32))
    decoder_hidden = np.ascontiguousarray(np.asarray(decoder_hidden, dtype=np.float32))

    if "nc" not in _NC_CACHE:
        _NC_CACHE["nc"] = _build_nc()
    nc = _NC_CACHE["nc"]

    in_maps = [
        {"enc": encoder_out[c * S_LOC : (c + 1) * S_LOC], "dec": decoder_hidden}
        for c in range(N_CORES)
    ]
    res = run_bass_kernel_spmd(nc, in_maps, core_ids=list(range(N_CORES)))
    LAST_RESULT = res

    parts = []
    for r in res.results:
        sc = np.asarray(r["out"])  # [128, 17]
        blk = np.concatenate(
            [
                sc[:, 0:14],
                (sc[:, 14] + sc[:, 15])[:, None],   # block 14
                sc[:, 16:17],                        # block 15
            ],
            axis=1,
        )  # [128, 16]
        parts.append(blk.T.reshape(-1))
    return np.concatenate(parts).astype(np.float32)


# revision 4
# speedup vs baseline: 1.2181x; 1.0029x over previous
"""Trainium2 Bass kernel: matvec, bf16 cast-on-DMA, PE t-broadcast via
PSUM + one-time ACT copy to SBUF, tapered ACT/DVE tail (v9).

scores = encoder_out[16384, 4096] @ decoder_hidden[-1][4096] -> [16384]
Sharding: encoder_out row-wise across 8 cores (2048 rows each),
decoder_hidden replicated; no cross-core communication.

Structure (per core, 32 MB fp32 read -> 16 MB bf16 in SBUF):
  - SWDGE (gpsimd) dma_start stream with fp32 -> bf16 cast in flight.
    SWDGE also avoids the HWDGE engine-15 descriptor-rate penalty in
    most runs (engine 15 has an environmental slow mode either way).
  - t broadcast to all 128 partitions with cast, also SWDGE (1 MB).
  - Blocks b0..b13: [128, 4096] tiles (rows n*128+p), 8 buffer slots.
  - Tail taper: b14 as two [128, 2048] halves reduced by ACT;
    b15 as four [128, 1024] quarters multiplied and reduced by DVE
    (reduce_sum into [128,1] scratch, 3 adds combine) so ACT and DVE
    drain the tail in parallel.
  - DVE tensor_mul in place (bf16), ACT Copy+accum_out does the row
    sums (fp32 accumulator).
  - Stores: sc[:, 0:12] early (descriptor generation overlaps the
    stream), sc[:, 12:17] behind the last reduce only.
  - Slot sems are reused for the tail tiles (each sem's final transfer
    makes the cumulative wait exact): 13 semaphores total keeps the
    preamble sem-init short.

Output sc [128, 17]: cols 0..13 = b0..b13 scores; block14 score =
sc[:,14] + sc[:,15] (the two halves); block15 score = sc[:,16].

Accuracy: enc and t are rounded to bf16 (products bf16, fp32
accumulate) -> max rel err ~3.4e-3, well under the 2e-2 gate.
"""

import numpy as np

S, H, L = 16384, 4096, 2
N_CORES = 8
S_LOC = S // N_CORES        # 2048
P = 128
N_BLOCKS = S_LOC // P       # 16
NBUF = 8
HH = H // 2                 # 2048
QW = H // 4                 # 1024

_NC_CACHE = {}
LAST_RESULT = None


def _build_nc():
    import concourse.bass as bass
    from concourse import mybir

    f32 = mybir.dt.float32
    bf16 = mybir.dt.bfloat16

    nc = bass.Bass(trn_type="TRN2")
    enc = nc.dram_tensor("enc", [S_LOC, H], f32, kind="ExternalInput")
    dec = nc.dram_tensor("dec", [L, H], f32, kind="ExternalInput")
    out = nc.dram_tensor("out", [P, 17], f32, kind="ExternalOutput")

    enc_r = enc.rearrange("(n p) h -> n p h", p=P)

    from contextlib import ExitStack

    with ExitStack() as ctx:
        t16 = ctx.enter_context(nc.sbuf_tensor("t16", [1, H], bf16))
        ones = ctx.enter_context(nc.sbuf_tensor("ones", [1, P], bf16))
        tbps = ctx.enter_context(nc.psum_tensor("tbps", [P, H], f32))
        tb = ctx.enter_context(nc.sbuf_tensor("tb", [P, H], bf16))
        ebufs = [
            ctx.enter_context(nc.sbuf_tensor(f"ebuf{i}", [P, H], bf16))
            for i in range(NBUF)
        ]
        junk = ctx.enter_context(nc.sbuf_tensor("junk", [P, H], bf16))
        sc = ctx.enter_context(nc.sbuf_tensor("sc", [P, 17], f32))
        scb = [
            ctx.enter_context(nc.sbuf_tensor(f"scb{k}", [P, 1], f32))
            for k in range(4)
        ]
        t_sem = ctx.enter_context(nc.semaphore("t_sem"))
        ones_sem = ctx.enter_context(nc.semaphore("ones_sem"))
        pe_sem = ctx.enter_context(nc.semaphore("pe_sem"))
        tb_sem = ctx.enter_context(nc.semaphore("tb_sem"))
        esems = [ctx.enter_context(nc.semaphore(f"esem{i}")) for i in range(NBUF)]
        hsems = [ctx.enter_context(nc.semaphore(f"hsem{i}")) for i in range(2)]
        qsems = [ctx.enter_context(nc.semaphore(f"qsem{i}")) for i in range(4)]
        mul_sem = ctx.enter_context(nc.semaphore("mul_sem"))
        red_sem = ctx.enter_context(nc.semaphore("red_sem"))
        qred_sem = ctx.enter_context(nc.semaphore("qred_sem"))
        store_sem = ctx.enter_context(nc.semaphore("store_sem"))
        block = ctx.enter_context(nc.Block())

        @block.sync
        def _(sync):
            # bulk store early: HWDGE descriptor generation (~128 descs)
            # overlaps the stream; only cols 12:17 wait for the tail
            sync.wait_ge(red_sem, 12)
            sync.dma_start(out[:, 0:12], sc[:, 0:12]).then_inc(store_sem, 16)
            sync.wait_ge(red_sem, 16)
            sync.wait_ge(qred_sem, 1)
            sync.dma_start(out[:, 12:17], sc[:, 12:17]).then_inc(store_sem, 16)
            sync.wait_ge(store_sem, 32)

        @block.gpsimd
        def _(gpsimd):
            # t into one partition with fp32 -> bf16 cast (8 KB written);
            # the idle PE broadcasts it to all 128 partitions via a
            # ones-vector matmul into PSUM -- saves ~5-6 us of broadcast
            # descriptors on every DMA engine
            gpsimd.memset(ones[:], 1.0).then_inc(ones_sem, 1)
            gpsimd.dma_start(t16[:], dec[L - 1 : L, :]).then_inc(t_sem, 16)
            # b0..b13 full tiles
            for i in range(N_BLOCKS - 2):
                if i >= NBUF:
                    gpsimd.wait_ge(red_sem, i - NBUF + 1)
                gpsimd.dma_start(ebufs[i % NBUF][:], enc_r[i]).then_inc(
                    esems[i % NBUF], 16
                )
            # b14 halves into slot 6 (b6's ACT frees it)
            gpsimd.wait_ge(red_sem, 7)
            gpsimd.dma_start(
                ebufs[6][:, 0:HH], enc_r[14, :, 0:HH]
            ).then_inc(hsems[0], 16)
            gpsimd.dma_start(
                ebufs[6][:, HH:H], enc_r[14, :, HH:H]
            ).then_inc(hsems[1], 16)
            # b15 quarters into slot 7 (b7's ACT frees it)
            gpsimd.wait_ge(red_sem, 8)
            for k in range(4):
                gpsimd.dma_start(
                    ebufs[7][:, k * QW : (k + 1) * QW],
                    enc_r[15, :, k * QW : (k + 1) * QW],
                ).then_inc(qsems[k], 16)

        @block.tensor
        def _(tensor):
            tensor.wait_ge(ones_sem, 1)
            tensor.wait_ge(t_sem, 16)
            for j in range(8):
                mm = nc.tensor.matmul(
                    tbps[:, j * 512 : (j + 1) * 512],
                    ones[:],
                    t16[:, j * 512 : (j + 1) * 512],
                    start=True,
                    stop=True,
                )
            mm.then_inc(pe_sem, 1)

        @block.vector
        def _(vector):
            vector.wait_ge(tb_sem, 1)
            for n in range(N_BLOCKS - 2):
                vector.wait_ge(esems[n % NBUF], 16 * (n // NBUF + 1))
                eb = ebufs[n % NBUF][:]
                nc.vector.tensor_mul(eb, eb, tb[:]).then_inc(mul_sem, 1)
            # b14 halves (ACT reduces them)
            vector.wait_ge(hsems[0], 16)
            nc.vector.tensor_mul(
                ebufs[6][:, 0:HH], ebufs[6][:, 0:HH], tb[:, 0:HH]
            ).then_inc(mul_sem, 1)
            vector.wait_ge(hsems[1], 16)
            nc.vector.tensor_mul(
                ebufs[6][:, HH:H], ebufs[6][:, HH:H], tb[:, HH:H]
            ).then_inc(mul_sem, 1)
            # b15 quarters: DVE multiplies AND reduces (ACT is busy with
            # the halves); 3 adds combine the 4 partials
            e7 = ebufs[7]
            for k in range(4):
                vector.wait_ge(qsems[k], 16)
                nc.vector.tensor_mul(
                    e7[:, k * QW : (k + 1) * QW],
                    e7[:, k * QW : (k + 1) * QW],
                    tb[:, k * QW : (k + 1) * QW],
                )
                nc.vector.reduce_sum(
                    out=scb[k][:],
                    in_=e7[:, k * QW : (k + 1) * QW],
                    axis=mybir.AxisListType.X,
                )
            nc.vector.tensor_add(scb[0][:], scb[0][:], scb[1][:])
            nc.vector.tensor_add(scb[2][:], scb[2][:], scb[3][:])
            # two junk spacer ops: the DVE pipeline has no RAW interlock at
            # 1-instruction distance (~70 ns); the final add must read
            # scb[2] at distance >= 3 (measured: distance-1 reads stale)
            nc.vector.tensor_add(scb[1][:], scb[1][:], scb[3][:])
            nc.vector.tensor_add(scb[3][:], scb[3][:], scb[1][:])
            # write sc[:, 16] only after ACT has finished its sc writes
            vector.wait_ge(red_sem, 16)
            nc.vector.tensor_add(
                sc[:, 16:17], scb[0][:], scb[2][:]
            ).then_inc(qred_sem, 1)

        @block.scalar
        def _(scalar):
            # warm the ACT function table while idle
            nc.scalar.activation(
                out=junk[0:1, 0:1],
                in_=junk[0:1, 0:1],
                func=mybir.ActivationFunctionType.Copy,
            )
            scalar.wait_ge(pe_sem, 1)
            nc.scalar.activation(
                out=tb[:],
                in_=tbps[:],
                func=mybir.ActivationFunctionType.Copy,
            ).then_inc(tb_sem, 1)
            for n in range(N_BLOCKS - 2):
                scalar.wait_ge(mul_sem, n + 1)
                nc.scalar.activation(
                    out=junk[:],
                    in_=ebufs[n % NBUF][:],
                    func=mybir.ActivationFunctionType.Copy,
                    accum_out=sc[:, n : n + 1],
                ).then_inc(red_sem, 1)
            # b14 halves -> sc cols 14, 15
            scalar.wait_ge(mul_sem, 15)
            nc.scalar.activation(
                out=junk[:, 0:HH],
                in_=ebufs[6][:, 0:HH],
                func=mybir.ActivationFunctionType.Copy,
                accum_out=sc[:, 14:15],
            ).then_inc(red_sem, 1)
            scalar.wait_ge(mul_sem, 16)
            nc.scalar.activation(
                out=junk[:, HH:H],
                in_=ebufs[6][:, HH:H],
                func=mybir.ActivationFunctionType.Copy,
                accum_out=sc[:, 15:16],
            ).then_inc(red_sem, 1)

    return nc


def kernel(encoder_out: np.ndarray, decoder_hidden: np.ndarray) -> np.ndarray:
    global LAST_RESULT
    from concourse.bass_utils import run_bass_kernel_spmd

    encoder_out = np.ascontiguousarray(np.asarray(encoder_out, dtype=np.float32))
    decoder_hidden = np.ascontiguousarray(np.asarray(decoder_hidden, dtype=np.float32))

    if "nc" not in _NC_CACHE:
        _NC_CACHE["nc"] = _build_nc()
    nc = _NC_CACHE["nc"]

    in_maps = [
        {"enc": encoder_out[c * S_LOC : (c + 1) * S_LOC], "dec": decoder_hidden}
        for c in range(N_CORES)
    ]
    res = run_bass_kernel_spmd(nc, in_maps, core_ids=list(range(N_CORES)))
    LAST_RESULT = res

    parts = []
    for r in res.results:
        sc = np.asarray(r["out"])  # [128, 17]
        blk = np.concatenate(
            [
                sc[:, 0:14],
                (sc[:, 14] + sc[:, 15])[:, None],   # block 14
                sc[:, 16:17],                        # block 15
            ],
            axis=1,
        )  # [128, 16]
        parts.append(blk.T.reshape(-1))
    return np.concatenate(parts).astype(np.float32)
